# revision 1
# baseline (speedup 1.0000x reference)
"""VQ codebook argmin kernel for Trainium2 (8 NeuronCores, SPMD).

Problem: feats [4,16,112,112] f32, vertex_embeddings [27554,16] f32.
Output: (feats unchanged, dps [4,112,112] int32) where
  dps[b,h,w] = argmin_v ||feats[b,:,h,w] - E[v]||^2.

argmin_v dist = argmax_v score, score_v = 2*x.e_v - ||e_v||^2 (exact fp32).

Strategy (data-parallel over pixels, 8 cores, 6272 px/core):
  - One matmul with augmented contraction K=17: lhsT = [2*x^T; ones] (pixels as
    the stationary 128-col operand), rhs = [E^T; -c2] chunk -> PSUM holds the
    score tile [128 px, 512 v] directly.
  - Vertex axis padded 27554 -> 28672 = 14 chunks x 2048 (padding scores -1e30).
  - Per chunk [128, 2048] in PSUM: vector.max (top8) + vector.max_index
    (within-chunk idx) into per-slab buffers.
  - Merge per slab: global max over the 14x8 top8 buffer, position via
    max_index, one-hot select of the stored within-chunk index, then
    global_idx = pos*256 + within. Single DMA of all 49 slab results at end.

A post-pass legalizes semaphore waits: cayman's EVENTS struct has ONE sync-wait
slot per instruction; Tile's sem assignment is not transitively minimal and can
emit same-engine waits (redundant: engines execute in order) plus the kernel
tail drain that joins every engine/DMA sem (engines are joined by the EVSEM
butterfly right after; consumed DMA-queue sems are provably complete).
"""

import sys

if "/opt/trn_rl_repo" not in sys.path:
    sys.path.insert(0, "/opt/trn_rl_repo")

import numpy as np

import concourse.bass as bass
import concourse.mybir as mybir
from concourse.bass_utils import run_bass_kernel_spmd
from concourse.tile import TileContext

# problem geometry (hardcoded per contest contract)
B, D, H, W_IMG = 4, 16, 112, 112
V = 27554
N_CORES = 8
NPIX = B * H * W_IMG  # 50176
COREPIX = NPIX // N_CORES  # 6272

P = 128  # pixels per slab (PSUM partition dim)
NSLAB = COREPIX // P  # 49
CHUNK = 2048  # vertices per PSUM chunk (4 banks)
NCHUNK = 14
VPAD = NCHUNK * CHUNK  # 28672
NMM = CHUNK // 512  # fp32 moving-operand max is 512
K = D + 1  # augmented contraction: 16 dims + bias row
NPOS = NCHUNK * 8  # top8-buffer width

NEG = -1.0e30


def _build():
    nc = bass.Bass()
    # packed input: [K, VPAD (codebook cols) | COREPIX (pixel cols) | NPOS (iota row 0)]
    WTOT = VPAD + COREPIX + NPOS
    vx = nc.dram_tensor("vx", [K, WTOT], mybir.dt.float32, kind="ExternalInput")
    out = nc.dram_tensor("dps", [P, NSLAB], mybir.dt.int32, kind="ExternalOutput")

    with TileContext(nc) as tc:
        with (
            tc.tile_pool(name="const", bufs=1) as cpool,
            tc.tile_pool(name="psum", bufs=2, space="PSUM") as ppool,
            tc.tile_pool(name="work", bufs=2) as wpool,
        ):
            vx_sb = cpool.tile([K, WTOT], mybir.dt.float32)
            nc.sync.dma_start(vx_sb[:, :], vx[:, :])
            vt_sb = vx_sb[:, :VPAD]
            xs_sb = vx_sb[:, VPAD : VPAD + COREPIX]
            iota_f = cpool.tile([P, NPOS], mybir.dt.float32)
            nc.sync.dma_start(
                iota_f[:, :], vx[0:1, VPAD + COREPIX :].to_broadcast([P, NPOS])
            )
            oacc = cpool.tile([P, NSLAB], mybir.dt.int32)

            for s in range(NSLAB):
                lhsT = xs_sb[:, s * P : (s + 1) * P]
                vbuf = wpool.tile([P, NPOS], mybir.dt.float32, tag="vbuf")
                ubuf = wpool.tile([P, NPOS], mybir.dt.uint16, tag="ubuf")
                for c in range(NCHUNK):
                    psc = ppool.tile([P, CHUNK], mybir.dt.float32, tag="psc")
                    for j in range(NMM):
                        nc.tensor.matmul(
                            psc[:, j * 512 : (j + 1) * 512],
                            lhsT,
                            vt_sb[:, c * CHUNK + j * 512 : c * CHUNK + (j + 1) * 512],
                            start=True,
                            stop=True,
                        )
                    nc.vector.max(out=vbuf[:, c * 8 : (c + 1) * 8], in_=psc[:, :])
                    nc.vector.max_index(
                        out=ubuf[:, c * 8 : (c + 1) * 8],
                        in_max=vbuf[:, c * 8 : (c + 1) * 8],
                        in_values=psc[:, :],
                    )
                gv = wpool.tile([P, 8], mybir.dt.float32, tag="gv")
                gpos = wpool.tile([P, 8], mybir.dt.uint16, tag="gpos")
                nc.vector.max(out=gv[:, :], in_=vbuf[:, :])
                nc.vector.max_index(out=gpos[:, :], in_max=gv[:, :], in_values=vbuf[:, :])
                gpos_f = wpool.tile([P, 1], mybir.dt.float32, tag="gposf")
                nc.vector.tensor_copy(gpos_f[:, :], gpos[:, 0:1])
                onehot = wpool.tile([P, NPOS], mybir.dt.float32, tag="onehot")
                nc.vector.tensor_scalar(
                    onehot[:, :],
                    iota_f[:, :],
                    gpos_f[:, 0:1],
                    None,
                    op0=mybir.AluOpType.is_equal,
                )
                nc.vector.tensor_tensor(
                    out=onehot[:, :],
                    in0=onehot[:, :],
                    in1=ubuf[:, :],
                    op=mybir.AluOpType.mult,
                )
                within = wpool.tile([P, 1], mybir.dt.float32, tag="within")
                nc.vector.reduce_sum(
                    within[:, :], onehot[:, :], axis=mybir.AxisListType.X
                )
                idxf = wpool.tile([P, 1], mybir.dt.float32, tag="idxf")
                nc.vector.tensor_scalar(
                    idxf[:, :],
                    gpos_f[:, 0:1],
                    float(CHUNK // 8),
                    None,
                    op0=mybir.AluOpType.mult,
                )
                nc.vector.tensor_tensor(
                    out=idxf[:, :],
                    in0=idxf[:, :],
                    in1=within[:, :],
                    op=mybir.AluOpType.add,
                )
                nc.vector.tensor_copy(oacc[:, s : s + 1], idxf[:, :])
            nc.sync.dma_start(out[:, :], oacc[:, :])
    _fix_sync_waits(nc)
    return nc


def _fix_sync_waits(nc):
    """Enforce cayman's one-sync-wait-per-instruction limit (see module doc)."""
    f = nc.m.functions[0]
    insts = [i for blk in f.blocks for i in blk.instructions]
    consumed = set()
    for inst in insts:
        si = inst.sync_info
        if si is None:
            continue
        for w in si.on_wait:
            consumed.add(w.ant_name)

    eng_sem_prefix = {
        mybir.EngineType.PE: "PE",
        mybir.EngineType.DVE: "DVE",
        mybir.EngineType.Activation: "Activation",
        mybir.EngineType.Pool: "Pool",
        mybir.EngineType.SP: "SP",
    }
    for inst in insts:
        si = inst.sync_info
        if si is None or len(si.on_wait) <= 1:
            continue
        if type(inst).__name__ == "InstDrain":
            keep = [
                w
                for w in si.on_wait
                if w.ant_name.startswith("DMA") and w.ant_name not in consumed
            ]
        else:
            pfx = eng_sem_prefix.get(inst.engine)
            keep = [
                w
                for w in si.on_wait
                if pfx is None or not w.ant_name.startswith(pfx + "_")
            ]
        assert len(keep) <= 1, (
            f"{inst.name} ({type(inst).__name__}): >1 wait after legalize: {keep}"
        )
        si.on_wait = keep
        inst.sync_info = si


_NC_CACHE = None


def _get_nc():
    global _NC_CACHE
    if _NC_CACHE is None:
        _NC_CACHE = _build()
    return _NC_CACHE


def _make_inputs(feats, vertex_embeddings):
    feats = np.ascontiguousarray(feats, dtype=np.float32)
    E = np.ascontiguousarray(vertex_embeddings, dtype=np.float32)
    # pixels as [NPIX, 16], pixel p = b*12544 + h*112 + w
    X = feats.reshape(B, D, H * W_IMG).transpose(0, 2, 1).reshape(NPIX, D)
    c2 = (E * E).sum(axis=1, dtype=np.float32)

    vt = np.full((K, VPAD), 0.0, np.float32)
    vt[:D, :V] = E.T
    vt[D, :V] = -c2
    vt[D, V:] = NEG

    iot = np.zeros((K, NPOS), np.float32)
    iot[0, :] = np.arange(NPOS, dtype=np.float32)

    in_maps = []
    for k in range(N_CORES):
        xs = np.empty((K, COREPIX), np.float32)
        xs[:D, :] = 2.0 * X[k * COREPIX : (k + 1) * COREPIX].T
        xs[D, :] = 1.0
        in_maps.append({"vx": np.concatenate([vt, xs, iot], axis=1)})
    return in_maps


def kernel(feats, vertex_embeddings, _trace=False):
    in_maps = _make_inputs(feats, vertex_embeddings)
    nc = _get_nc()
    res = run_bass_kernel_spmd(
        nc, in_maps, core_ids=list(range(N_CORES)), trace=_trace
    )
    parts = []
    for k in range(N_CORES):
        dk = res.results[k]["dps"]  # [P, NSLAB] int32
        parts.append(dk.T.reshape(-1))  # slab-major -> pixel order
    dps = np.concatenate(parts).reshape(B, H, W_IMG).astype(np.int32)
    feats_out = np.ascontiguousarray(feats, dtype=np.float32)
    kernel._last_results = res
    return feats_out, dps


# revision 4
# speedup vs baseline: 361.6787x; 361.6787x over previous
"""VQ codebook argmin kernel for Trainium2 (8 NeuronCores, SPMD), two-phase.

Problem: feats [4,16,112,112] f32, vertex_embeddings [27554,16] f32.
Output: (feats unchanged, dps [4,112,112] int32) where
  dps[b,h,w] = argmin_v ||feats[b,:,h,w] - E[v]||^2.

argmin_v dist = argmax_v score, score_v = 2*x.e_v - ||e_v||^2.

Data-parallel over pixels (8 cores x 6272 px). Vertex axis padded to
28672 = 14 chunks x 2048 (padding scores -1e30).

Phase A (fast, reduced precision): float32r matmuls (full PE rate; hardware
truncates operands to ~fp22, |score_f32r - score_fp32| <= DELTA empirically
with large margin) compute score chunks in PSUM; one reduce_max per chunk
gives each pixel's per-chunk maximum m~[px, 14]. No index extraction -- this
halves the DVE scan versus a max+max_index design, and the DVE scan is the
kernel's floor.

Host: candidate chunks per pixel = {c : m~_c >= max_c m~ - 2*DELTA}. The true
fp32 argmax provably lies in a candidate chunk. Pixels are binned per-core by
candidate chunk into fixed-capacity slabs (static NEFF layout; the slab ->
chunk map is compile-time constant).

Phase B (exact): for each bin slab, fp32 matmuls recompute the chunk's scores
exactly; vector.max + max_index give (m, within). Host merges candidates per
pixel (max m, ties -> lower chunk, i.e. lowest global index, matching
jnp.argmin semantics) and assembles dps = c*2048 + within.

A post-pass legalizes semaphore waits: cayman's EVENTS struct has ONE
sync-wait slot per instruction; Tile's sem assignment is not transitively
minimal and can emit same-engine waits (redundant: engines execute in order)
plus a kernel-tail drain joining every engine/DMA sem (engines are joined by
the EVSEM butterfly right after it; DMA-queue sems already waited on by some
instruction are provably complete).
"""

import sys

if "/opt/trn_rl_repo" not in sys.path:
    sys.path.insert(0, "/opt/trn_rl_repo")

import numpy as np

import concourse.bass as bass
import concourse.mybir as mybir
from concourse.bass_utils import run_bass_kernel_spmd
from concourse.tile import TileContext

# problem geometry (hardcoded per contest contract)
B, D, H, W_IMG = 4, 16, 112, 112
V = 27554
N_CORES = 8
NPIX = B * H * W_IMG  # 50176
COREPIX = NPIX // N_CORES  # 6272

P = 128
NSLAB = COREPIX // P  # 49
CHUNK = 2048
NCHUNK = 14
VPAD = NCHUNK * CHUNK  # 28672
NMM = CHUNK // 512
K = D + 1  # augmented contraction: 16 dims + bias row for -|e|^2

NEG = -1.0e30
# |score_float32r - score_fp32| bound: emulated-fp22 full-data max is 0.062,
# measured HW error is ~4x smaller; 0.10 gives >3x margin over expected HW max.
DELTA = 0.10
NB = 6  # phase-B slabs per chunk-bin (capacity 768 px/bin vs ~450 mean)
NSLAB2 = NCHUNK * NB  # 84


def _fix_sync_waits(nc):
    """Enforce cayman's one-sync-wait-per-instruction limit (see module doc)."""
    f = nc.m.functions[0]
    insts = [i for blk in f.blocks for i in blk.instructions]
    consumed = set()
    for inst in insts:
        si = inst.sync_info
        if si is None:
            continue
        for w in si.on_wait:
            consumed.add(w.ant_name)

    eng_sem_prefix = {
        mybir.EngineType.PE: "PE",
        mybir.EngineType.DVE: "DVE",
        mybir.EngineType.Activation: "Activation",
        mybir.EngineType.Pool: "Pool",
        mybir.EngineType.SP: "SP",
    }
    for inst in insts:
        si = inst.sync_info
        if si is None or len(si.on_wait) <= 1:
            continue
        if type(inst).__name__ == "InstDrain":
            keep = [
                w
                for w in si.on_wait
                if w.ant_name.startswith("DMA") and w.ant_name not in consumed
            ]
        else:
            pfx = eng_sem_prefix.get(inst.engine)
            keep = [
                w
                for w in si.on_wait
                if pfx is None or not w.ant_name.startswith(pfx + "_")
            ]
        assert len(keep) <= 1, (
            f"{inst.name} ({type(inst).__name__}): >1 wait after legalize: {keep}"
        )
        si.on_wait = keep
        inst.sync_info = si


def _build_phase_a():
    nc = bass.Bass()
    WTOT = VPAD + COREPIX
    vx = nc.dram_tensor("vx", [K, WTOT], mybir.dt.float32r, kind="ExternalInput")
    out = nc.dram_tensor(
        "cm", [P, NSLAB * NCHUNK], mybir.dt.float32, kind="ExternalOutput"
    )
    with TileContext(nc) as tc:
        with (
            tc.tile_pool(name="const", bufs=1) as cpool,
            tc.tile_pool(name="psum", bufs=2, space="PSUM") as ppool,
        ):
            vx_sb = cpool.tile([K, WTOT], mybir.dt.float32r)
            nc.sync.dma_start(vx_sb[:, :], vx[:, :])
            vt_sb = vx_sb[:, :VPAD]
            xs_sb = vx_sb[:, VPAD:]
            obuf = cpool.tile([P, NSLAB * NCHUNK], mybir.dt.float32)
            for s in range(NSLAB):
                lhsT = xs_sb[:, s * P : (s + 1) * P]
                for c in range(NCHUNK):
                    psc = ppool.tile([P, CHUNK], mybir.dt.float32, tag="psc")
                    for j in range(NMM):
                        nc.tensor.matmul(
                            psc[:, j * 512 : (j + 1) * 512],
                            lhsT,
                            vt_sb[:, c * CHUNK + j * 512 : c * CHUNK + (j + 1) * 512],
                            start=True,
                            stop=True,
                        )
                    nc.vector.reduce_max(
                        obuf[:, s * NCHUNK + c : s * NCHUNK + c + 1],
                        psc[:, :],
                        axis=mybir.AxisListType.X,
                    )
            nc.sync.dma_start(out[:, :], obuf[:, :])
    _fix_sync_waits(nc)
    return nc


def _build_phase_b():
    nc = bass.Bass()
    WTOT = VPAD + NSLAB2 * P
    vx = nc.dram_tensor("vx", [K, WTOT], mybir.dt.float32, kind="ExternalInput")
    out = nc.dram_tensor("res", [P, NSLAB2 * 2], mybir.dt.float32, kind="ExternalOutput")
    with TileContext(nc) as tc:
        with (
            tc.tile_pool(name="const", bufs=1) as cpool,
            tc.tile_pool(name="psum", bufs=2, space="PSUM") as ppool,
            tc.tile_pool(name="work", bufs=2) as wpool,
        ):
            vx_sb = cpool.tile([K, WTOT], mybir.dt.float32)
            nc.sync.dma_start(vx_sb[:, :], vx[:, :])
            vt_sb = vx_sb[:, :VPAD]
            xs_sb = vx_sb[:, VPAD:]
            obuf = cpool.tile([P, NSLAB2 * 2], mybir.dt.float32)
            for g in range(NSLAB2):
                c = g // NB
                lhsT = xs_sb[:, g * P : (g + 1) * P]
                psc = ppool.tile([P, CHUNK], mybir.dt.float32, tag="psc")
                for j in range(NMM):
                    nc.tensor.matmul(
                        psc[:, j * 512 : (j + 1) * 512],
                        lhsT,
                        vt_sb[:, c * CHUNK + j * 512 : c * CHUNK + (j + 1) * 512],
                        start=True,
                        stop=True,
                    )
                mt = wpool.tile([P, 8], mybir.dt.float32, tag="mt")
                ut = wpool.tile([P, 8], mybir.dt.uint16, tag="ut")
                nc.vector.max(out=mt[:, :], in_=psc[:, :])
                nc.vector.max_index(out=ut[:, :], in_max=mt[:, :], in_values=psc[:, :])
                nc.vector.tensor_copy(obuf[:, 2 * g : 2 * g + 1], mt[:, 0:1])
                nc.vector.tensor_copy(obuf[:, 2 * g + 1 : 2 * g + 2], ut[:, 0:1])
            nc.sync.dma_start(out[:, :], obuf[:, :])
    _fix_sync_waits(nc)
    return nc


_NC_A = None
_NC_B = None


def _get_ncs():
    global _NC_A, _NC_B
    if _NC_A is None:
        _NC_A = _build_phase_a()
        _NC_B = _build_phase_b()
    return _NC_A, _NC_B


def kernel(feats, vertex_embeddings, _trace=False):
    feats_in = feats
    feats = np.ascontiguousarray(feats, dtype=np.float32)
    E = np.ascontiguousarray(vertex_embeddings, dtype=np.float32)
    X = feats.reshape(B, D, H * W_IMG).transpose(0, 2, 1).reshape(NPIX, D)
    c2 = (E * E).sum(axis=1, dtype=np.float32)

    vt = np.zeros((K, VPAD), np.float32)
    vt[:D, :V] = E.T
    vt[D, :V] = -c2
    vt[D, V:] = NEG

    # per-core pixel columns [2x; 1]
    xcols = np.empty((K, NPIX), np.float32)
    xcols[:D, :] = 2.0 * X.T
    xcols[D, :] = 1.0

    nc_a, nc_b = _get_ncs()

    # ---- phase A ----
    in_maps_a = [
        {"vx": np.concatenate([vt, xcols[:, k * COREPIX : (k + 1) * COREPIX]], axis=1)}
        for k in range(N_CORES)
    ]
    res_a = run_bass_kernel_spmd(nc_a, in_maps_a, core_ids=list(range(N_CORES)))

    # ---- host: candidate chunks + binning ----
    in_maps_b = []
    slot_pix = np.full((N_CORES, NSLAB2 * P), -1, np.int64)  # slot -> core-local pixel
    for k in range(N_CORES):
        cm = res_a.results[k]["cm"]  # [P, NSLAB*NCHUNK]
        m = (
            cm.reshape(P, NSLAB, NCHUNK)
            .transpose(1, 0, 2)
            .reshape(COREPIX, NCHUNK)
        )
        mmax = m.max(axis=1)
        cand = m >= (mmax[:, None] - 2.0 * DELTA)  # [COREPIX, 14]
        xg = np.zeros((K, NSLAB2 * P), np.float32)
        base = k * COREPIX
        for c in range(NCHUNK):
            pix = np.nonzero(cand[:, c])[0]
            assert pix.size <= NB * P, (
                f"phase-B bin overflow: core {k} chunk {c}: {pix.size} > {NB * P}"
            )
            lo = c * NB * P
            xg[:, lo : lo + pix.size] = xcols[:, base + pix]
            slot_pix[k, lo : lo + pix.size] = pix
        in_maps_b.append({"vx": np.concatenate([vt, xg], axis=1)})

    # ---- phase B ----
    res_b = run_bass_kernel_spmd(nc_b, in_maps_b, core_ids=list(range(N_CORES)))

    # ---- host: merge candidates ----
    dps = np.empty(NPIX, np.int32)
    for k in range(N_CORES):
        res = res_b.results[k]["res"]  # [P, NSLAB2*2]
        res = res.reshape(P, NSLAB2, 2).transpose(1, 0, 2).reshape(NSLAB2 * P, 2)
        sp = slot_pix[k]
        slots = np.nonzero(sp >= 0)[0]
        pix = sp[slots]
        chunk = slots // (NB * P)
        mvals = res[slots, 0]
        gidx = chunk * CHUNK + res[slots, 1].astype(np.int64)
        # per-pixel argmax over candidate slots; ties -> lowest chunk (= lowest
        # global index), matching argmin-first semantics: sort by (pixel asc,
        # m desc, chunk asc) and take the first slot per pixel.
        order = np.lexsort((chunk, -mvals.astype(np.float64), pix))
        pix_s = pix[order]
        first = np.unique(pix_s, return_index=True)[1]
        assert first.size == COREPIX, "pixel missing from phase-B candidates"
        dps[k * COREPIX : (k + 1) * COREPIX] = gidx[order][first]
    dps = dps.reshape(B, H, W_IMG).astype(np.int32)

    kernel._last_results = (res_a, res_b)
    feats_out = np.ascontiguousarray(feats_in, dtype=np.float32)
    return feats_out, dps
